# revision 1
# baseline (speedup 1.0000x reference)
"""InternLM2 decoder layer on 8 trn2 NeuronCores, tensor-parallel (bass/Tile).

Self-contained: hardcodes shapes/sharding. Host shards + pre-tiles weights
(bf16, RMSNorm gammas folded into consuming matmul weights), device computes
the layer, host reassembles the output.

Per-core sharding: q-heads 4c..4c+3 + kv-head c (GQA groups align), wo/w2
row-sharded, w1/w3 col-sharded, tokens 256c..256c+256 owned for norms and
residuals. Dataflow: slice-norm -> AllGather(xnT) -> QKV/attention/wo ->
ReduceScatter -> slice-norm -> AllGather -> MLP -> ReduceScatter -> residual.
Activations stay hid-major [k, t]; scores computed transposed [s, t] with
fixed-max softmax (scores bounded ~8 for this distribution), denominator via
ones-matmul, PV yields out_hT directly.
"""
import sys
import numpy as np
import ml_dtypes

sys.path.insert(0, "/opt/trn_rl_repo")

HID, H, K, D, INTER, T = 4096, 32, 8, 128, 14336, 2048
EPS, THETA = 1e-5, 1000000.0
NC = 8                 # cores
QH = H // NC           # q heads per core = 4
JD = QH * D            # per-core attn out dim = 512
IS = INTER // NC       # inter shard = 1792
TOK = T // NC          # owned tokens per core = 256
CH = 512               # token chunk for compute loops
NCH = T // CH          # 4
KB_ = HID // 128       # 32 k-tiles
IT_ = IS // 128        # 14 i-tiles
SCALE = 1.0 / np.sqrt(D)

bf16 = ml_dtypes.bfloat16

_compiled = None


def _build(collectives=True, repeat=1):
    from contextlib import ExitStack
    import concourse.bacc as bacc
    import concourse.bass as bass
    import concourse.tile as tile
    from concourse import mybir

    f32 = mybir.dt.float32
    bf = mybir.dt.bfloat16
    AF = mybir.ActivationFunctionType
    PSUM = bass.MemorySpace.PSUM

    nc = bacc.Bacc("TRN2", target_bir_lowering=False, debug=False, num_devices=NC)

    # ---- I/O (per-core shapes; weights pre-tiled on host) ----
    x_own = nc.dram_tensor("x_own", [TOK, HID], f32, kind="ExternalInput")
    cosT = nc.dram_tensor("cosT", [D // 2, T], f32, kind="ExternalInput")
    sinT = nc.dram_tensor("sinT", [D // 2, T], f32, kind="ExternalInput")
    ident = nc.dram_tensor("ident", [128, 128], bf, kind="ExternalInput")
    wqkvR = nc.dram_tensor("wqkvR", [128, KB_, JD + 2 * D], bf, kind="ExternalInput")
    woR = nc.dram_tensor("woR", [128, QH, HID], bf, kind="ExternalInput")
    w1R = nc.dram_tensor("w1R", [IT_, 128, KB_, 128], bf, kind="ExternalInput")
    w3R = nc.dram_tensor("w3R", [IT_, 128, KB_, 128], bf, kind="ExternalInput")
    w2R = nc.dram_tensor("w2R", [128, IT_, HID], bf, kind="ExternalInput")
    out_own = nc.dram_tensor("out_own", [TOK, HID], f32, kind="ExternalOutput")

    # ---- internal DRAM (collective bounce + h spill) ----
    ag1_in = nc.dram_tensor("ag1_in", [HID, TOK], bf, kind="Internal")
    ag1_out = nc.dram_tensor("ag1_out", [NC, HID, TOK], bf, kind="Internal",
                             addr_space="Shared")
    rs1_in = nc.dram_tensor("rs1_in", [T, HID], bf, kind="Internal")
    rs1_out = nc.dram_tensor("rs1_out", [TOK, HID], bf, kind="Internal")
    ag2_in = nc.dram_tensor("ag2_in", [HID, TOK], bf, kind="Internal")
    ag2_out = nc.dram_tensor("ag2_out", [NC, HID, TOK], bf, kind="Internal",
                             addr_space="Shared")
    rs2_in = nc.dram_tensor("rs2_in", [T, HID], bf, kind="Internal")
    rs2_out = nc.dram_tensor("rs2_out", [TOK, HID], bf, kind="Internal")
    h_spill = nc.dram_tensor("h_spill", [TOK, HID], f32, kind="Internal")

    RG = [list(range(NC))]

    def do_collective(kind, op, in_t, out_t):
        if collectives:
            nc.gpsimd.collective_compute(
                kind, op, replica_groups=RG, ins=[in_t.ap()], outs=[out_t.ap()])
        elif kind == "AllGather":
            nc.sync.dma_start(out_t.ap()[0], in_t.ap())
        else:
            nc.sync.dma_start(out_t.ap(), in_t.ap()[0:TOK, :])

    with tile.TileContext(nc) as tc, ExitStack() as top:
        const = top.enter_context(tc.tile_pool(name="const", bufs=1))
        ident_sb = const.tile([128, 128], bf)
        nc.sync.dma_start(ident_sb[:], ident.ap())
        ones_sb = const.tile([128, 1], bf)
        nc.vector.memset(ones_sb[:], 1.0)
        eps_sb = const.tile([128, 1], f32)
        nc.vector.memset(eps_sb[:], EPS)

        # ---- norm of [TOK, HID] f32 token-major dram -> transposed bf16 to
        # [HID, TOK] dram ----
        def slice_norm_transpose(ctx, src_dram, dst_dram):
            pool = ctx.enter_context(tc.tile_pool(name="norm", bufs=2))
            psum = ctx.enter_context(
                tc.tile_pool(name="normps", bufs=2, space=PSUM))
            for b in range(TOK // 128):
                xt = pool.tile([128, HID], f32, tag="xt")
                nc.sync.dma_start(xt[:], src_dram.ap()[b * 128:(b + 1) * 128, :])
                sq = pool.tile([128, HID], bf, tag="sq")
                ssq = pool.tile([128, 1], f32, tag="ssq")
                nc.scalar.activation(sq[:], xt[:], AF.Square, accum_out=ssq[:])
                rms = pool.tile([128, 1], f32, tag="rms")
                nc.scalar.activation(rms[:], ssq[:], AF.Sqrt,
                                     scale=1.0 / HID, bias=eps_sb[:])
                rinv = pool.tile([128, 1], f32, tag="rinv")
                nc.vector.reciprocal(rinv[:], rms[:])
                xn = pool.tile([128, HID], bf, tag="xn")
                nc.vector.tensor_scalar_mul(xn[:], xt[:], rinv[:])
                for kb in range(KB_):
                    tp = psum.tile([128, 128], bf, tag="tp")
                    nc.tensor.transpose(tp[:], xn[:, kb * 128:(kb + 1) * 128],
                                        ident_sb[:])
                    tb = pool.tile([128, 128], bf, tag="tb")
                    nc.vector.tensor_copy(tb[:], tp[:])
                    nc.sync.dma_start(
                        dst_dram.ap()[kb * 128:(kb + 1) * 128,
                                      b * 128:(b + 1) * 128], tb[:])

        for _rep in range(repeat):
            # ================= phase 1: norm1 + AG1 =================
            with ExitStack() as ph:
                slice_norm_transpose(ph, x_own, ag1_in)
            do_collective("AllGather", mybir.AluOpType.bypass, ag1_in, ag1_out)

            # ================= phase 2: QKV + attention + wo =================
            with ExitStack() as ph:
                wpool = ph.enter_context(tc.tile_pool(name="wqkv", bufs=1))
                wqkv_sb = wpool.tile([128, KB_, JD + 2 * D], bf)
                nc.sync.dma_start(wqkv_sb[:], wqkvR.ap())
                wo_sb = wpool.tile([128, QH, HID], bf)
                nc.sync.dma_start(wo_sb[:], woR.ap())
                kv_pool = ph.enter_context(tc.tile_pool(name="kv", bufs=1))
                kT_sb = kv_pool.tile([128, T], bf)            # roped K, [d, t]
                v_sb = kv_pool.tile([128, T // 128, D], bf)   # [d-part, s-tile, d]
                cos_sb = kv_pool.tile([D // 2, T], f32)
                sin_sb = kv_pool.tile([D // 2, T], f32)
                nc.sync.dma_start(cos_sb[:], cosT.ap())
                nc.sync.dma_start(sin_sb[:], sinT.ap())

                xc_pool = ph.enter_context(tc.tile_pool(name="attnxc", bufs=1))
                ap_ = ph.enter_context(tc.tile_pool(name="attn", bufs=2))
                mm_ps = ph.enter_context(tc.tile_pool(name="mmps", bufs=2, space=PSUM))
                pv_ps = ph.enter_context(tc.tile_pool(name="pvps", bufs=1, space=PSUM))
                wo_ps = ph.enter_context(tc.tile_pool(name="wops", bufs=1, space=PSUM))

                def rope(dst, src, t0):
                    c = cos_sb[:, t0:t0 + CH]
                    s = sin_sb[:, t0:t0 + CH]
                    t1 = ap_.tile([64, CH], f32, tag="rp1")
                    t2 = ap_.tile([64, CH], f32, tag="rp2")
                    nc.vector.tensor_mul(t1[:], src[0:64, :], c)
                    nc.vector.tensor_mul(t2[:], src[64:128, :], s)
                    nc.vector.tensor_sub(dst[0:64, :], t1[:], t2[:])
                    nc.vector.tensor_mul(t1[:], src[64:128, :], c)
                    nc.vector.tensor_mul(t2[:], src[0:64, :], s)
                    nc.vector.tensor_add(dst[64:128, :], t1[:], t2[:])

                for j in range(NCH):
                    t0 = j * CH
                    xc = xc_pool.tile([128, KB_, CH], bf, tag="xc")
                    for half in range(2):
                        nc.sync.dma_start(
                            xc[:, :, half * 256:(half + 1) * 256],
                            ag1_out.ap()[2 * j + half].rearrange(
                                "(a p) t -> p a t", p=128))
                    qT = ap_.tile([128, QH, CH], bf, tag="qT")
                    for m in range(6):
                        acc = mm_ps.tile([128, CH], f32, tag="mm")
                        for kb in range(KB_):
                            nc.tensor.matmul(
                                acc[:],
                                wqkv_sb[:, kb, m * 128:(m + 1) * 128],
                                xc[:, kb, :],
                                start=(kb == 0), stop=(kb == KB_ - 1))
                        if m < QH:
                            rope(qT[:, m, :], acc, t0)
                        elif m == QH:
                            rope(kT_sb[:, t0:t0 + CH], acc, t0)
                        else:
                            vb = ap_.tile([128, CH], bf, tag="vb")
                            nc.vector.tensor_copy(vb[:], acc[:])
                            for sb_ in range(CH // 128):
                                tp = mm_ps.tile([128, 128], bf, tag="vtp")
                                nc.tensor.transpose(
                                    tp[:], vb[:, sb_ * 128:(sb_ + 1) * 128],
                                    ident_sb[:])
                                nc.vector.tensor_copy(
                                    v_sb[:, t0 // 128 + sb_, :], tp[:])

                    aoT = ap_.tile([128, QH, CH], bf, tag="aoT")
                    for hq in range(QH):
                        pv = pv_ps.tile([128, CH], f32, tag="pv")
                        den = pv_ps.tile([1, CH], f32, tag="den")
                        ns = (t0 + CH) // 128
                        for si in range(ns):
                            sc = mm_ps.tile([128, CH], f32, tag="mm")
                            nc.tensor.matmul(sc[:], kT_sb[:, si * 128:(si + 1) * 128],
                                             qT[:, hq, :], start=True, stop=True)
                            pT = ap_.tile([128, CH], bf, tag="pT")
                            nc.scalar.activation(pT[:], sc[:], AF.Exp, scale=SCALE)
                            if si * 128 + 127 > t0:      # diagonal: zero s > t
                                pm = ap_.tile([128, CH], bf, tag="pm")
                                nc.gpsimd.affine_select(
                                    pm[:], pT[:], pattern=[[1, CH]],
                                    compare_op=mybir.AluOpType.is_ge,
                                    fill=0.0, base=t0 - si * 128,
                                    channel_multiplier=-1)
                                pT = pm
                            nc.tensor.matmul(pv[:], v_sb[:, si, :], pT[:],
                                             start=(si == 0), stop=(si == ns - 1))
                            nc.tensor.matmul(den[:], ones_sb[:], pT[:],
                                             start=(si == 0), stop=(si == ns - 1))
                        rec = ap_.tile([1, CH], f32, tag="rec")
                        nc.vector.reciprocal(rec[:], den[:])
                        recb = ap_.tile([128, CH], f32, tag="recb")
                        nc.gpsimd.partition_broadcast(recb[:], rec[:])
                        nc.vector.tensor_mul(aoT[:, hq, :], pv[:], recb[:])

                    # wo: out[t, hid], M=4x128, N=4096 (4 psum tiles of 1024), K=512
                    for m in range(CH // 128):
                        for nh in range(4):
                            acc = wo_ps.tile([128, 1024], f32, tag="wo")
                            for kb in range(QH):
                                for n2 in range(2):
                                    nc.tensor.matmul(
                                        acc[:, n2 * 512:(n2 + 1) * 512],
                                        aoT[:, kb, m * 128:(m + 1) * 128],
                                        wo_sb[:, kb, nh * 1024 + n2 * 512:
                                              nh * 1024 + (n2 + 1) * 512],
                                        start=(kb == 0), stop=(kb == QH - 1))
                            ob = ap_.tile([128, 1024], bf, tag="ob")
                            nc.vector.tensor_copy(ob[:], acc[:])
                            nc.sync.dma_start(
                                rs1_in.ap()[t0 + m * 128: t0 + (m + 1) * 128,
                                            nh * 1024:(nh + 1) * 1024], ob[:])

            do_collective("ReduceScatter", mybir.AluOpType.add, rs1_in, rs1_out)

            # ================= phase 3: h = x + rs1, norm2, AG2 =================
            with ExitStack() as ph:
                pool = ph.enter_context(tc.tile_pool(name="resid", bufs=2))
                for b in range(TOK // 128):
                    xt = pool.tile([128, HID], f32, tag="xt")
                    nc.sync.dma_start(xt[:], x_own.ap()[b * 128:(b + 1) * 128, :])
                    rt = pool.tile([128, HID], bf, tag="rt")
                    nc.sync.dma_start(rt[:], rs1_out.ap()[b * 128:(b + 1) * 128, :])
                    ht = pool.tile([128, HID], f32, tag="ht")
                    nc.vector.tensor_add(ht[:], xt[:], rt[:])
                    nc.sync.dma_start(h_spill.ap()[b * 128:(b + 1) * 128, :], ht[:])
            with ExitStack() as ph:
                slice_norm_transpose(ph, h_spill, ag2_in)
            do_collective("AllGather", mybir.AluOpType.bypass, ag2_in, ag2_out)

            # ================= phase 4: MLP =================
            with ExitStack() as ph:
                big = ph.enter_context(tc.tile_pool(name="mlpbig", bufs=1))
                mp = ph.enter_context(tc.tile_pool(name="mlp", bufs=2))
                wsp = ph.enter_context(tc.tile_pool(name="w13", bufs=2))
                gu_ps = ph.enter_context(tc.tile_pool(name="gups", bufs=2, space=PSUM))
                d_ps = ph.enter_context(tc.tile_pool(name="dps", bufs=2, space=PSUM))

                w2c = big.tile([128, IT_, HID], bf, tag="w2c")
                nc.sync.dma_start(w2c[:], w2R.ap())

                for j in range(NCH):
                    t0 = j * CH
                    xc = big.tile([128, KB_, CH], bf, tag="xc")
                    for half in range(2):
                        nc.sync.dma_start(
                            xc[:, :, half * 256:(half + 1) * 256],
                            ag2_out.ap()[2 * j + half].rearrange(
                                "(a p) t -> p a t", p=128))
                    actT = big.tile([128, IT_, CH], bf, tag="actT")
                    for it in range(IT_):
                        w1t = wsp.tile([128, KB_, 128], bf, tag="w1t")
                        w3t = wsp.tile([128, KB_, 128], bf, tag="w3t")
                        nc.sync.dma_start(w1t[:], w1R.ap()[it])
                        nc.sync.dma_start(w3t[:], w3R.ap()[it])
                        g = gu_ps.tile([128, CH], f32, tag="g")
                        u = gu_ps.tile([128, CH], f32, tag="u")
                        for kb in range(KB_):
                            nc.tensor.matmul(g[:], w1t[:, kb, :], xc[:, kb, :],
                                             start=(kb == 0), stop=(kb == KB_ - 1))
                        for kb in range(KB_):
                            nc.tensor.matmul(u[:], w3t[:, kb, :], xc[:, kb, :],
                                             start=(kb == 0), stop=(kb == KB_ - 1))
                        sg = mp.tile([128, CH], f32, tag="sg")
                        nc.scalar.activation(sg[:], g[:], AF.Silu)
                        nc.vector.tensor_mul(actT[:, it, :], sg[:], u[:])
                    # down-proj
                    for m in range(CH // 128):
                        for nh in range(4):
                            acc = d_ps.tile([128, 1024], f32, tag="d")
                            for it in range(IT_):
                                for n2 in range(2):
                                    nc.tensor.matmul(
                                        acc[:, n2 * 512:(n2 + 1) * 512],
                                        actT[:, it, m * 128:(m + 1) * 128],
                                        w2c[:, it, nh * 1024 + n2 * 512:
                                            nh * 1024 + (n2 + 1) * 512],
                                        start=(it == 0), stop=(it == IT_ - 1))
                            ob = mp.tile([128, 1024], bf, tag="ob")
                            nc.vector.tensor_copy(ob[:], acc[:])
                            nc.sync.dma_start(
                                rs2_in.ap()[t0 + m * 128: t0 + (m + 1) * 128,
                                            nh * 1024:(nh + 1) * 1024], ob[:])

            do_collective("ReduceScatter", mybir.AluOpType.add, rs2_in, rs2_out)

            # ================= phase 5: final residual =================
            with ExitStack() as ph:
                pool = ph.enter_context(tc.tile_pool(name="fin", bufs=2))
                for b in range(TOK // 128):
                    ht = pool.tile([128, HID], f32, tag="ht")
                    nc.sync.dma_start(ht[:], h_spill.ap()[b * 128:(b + 1) * 128, :])
                    rt = pool.tile([128, HID], bf, tag="rt")
                    nc.sync.dma_start(rt[:], rs2_out.ap()[b * 128:(b + 1) * 128, :])
                    ot = pool.tile([128, HID], f32, tag="ot")
                    nc.vector.tensor_add(ot[:], ht[:], rt[:])
                    nc.sync.dma_start(out_own.ap()[b * 128:(b + 1) * 128, :], ot[:])

    nc.compile()
    return nc


def _get_compiled():
    global _compiled
    if _compiled is None:
        _compiled = _build()
    return _compiled


def _prep_inputs(inputs):
    x = np.asarray(inputs["hidden_states"], np.float32)
    pos = np.asarray(inputs["position_ids"]).astype(np.float32)
    wqkv = np.asarray(inputs["wqkv"], np.float32)
    wo = np.asarray(inputs["wo"], np.float32)
    w1 = np.asarray(inputs["w1"], np.float32)
    w3 = np.asarray(inputs["w3"], np.float32)
    w2 = np.asarray(inputs["w2"], np.float32)
    anw = np.asarray(inputs["attn_norm_w"], np.float32)
    fnw = np.asarray(inputs["ffn_norm_w"], np.float32)

    inv_freq = 1.0 / (THETA ** (np.arange(0, D, 2, dtype=np.float32) / D))
    freqs = pos[:, None] * inv_freq
    cosT_np = np.ascontiguousarray(np.cos(freqs).T.astype(np.float32))
    sinT_np = np.ascontiguousarray(np.sin(freqs).T.astype(np.float32))
    ident_np = np.ascontiguousarray(np.eye(128, dtype=bf16))

    wqkv_f = wqkv * anw[None, :]
    w1_f = w1 * fnw[None, :]
    w3_f = w3 * fnw[None, :]

    def ktile_major(wT, n):           # [HID, n] -> [128, KB_, n]
        return np.ascontiguousarray(
            wT.reshape(KB_, 128, n).transpose(1, 0, 2).astype(bf16))

    in_maps = []
    for c in range(NC):
        qrows = np.arange(JD * c, JD * (c + 1))
        krows = H * D + np.arange(D * c, D * (c + 1))
        vrows = (H + K) * D + np.arange(D * c, D * (c + 1))
        rows = np.concatenate([qrows, krows, vrows])
        w1T = w1_f[IS * c:IS * (c + 1)].T          # [HID, IS]
        w3T = w3_f[IS * c:IS * (c + 1)].T
        in_maps.append({
            "x_own": np.ascontiguousarray(x[TOK * c:TOK * (c + 1)]),
            "cosT": cosT_np, "sinT": sinT_np, "ident": ident_np,
            "wqkvR": ktile_major(wqkv_f[rows].T, JD + 2 * D),
            "woR": np.ascontiguousarray(
                wo[:, JD * c:JD * (c + 1)].T.reshape(QH, 128, HID)
                .transpose(1, 0, 2).astype(bf16)),
            "w1R": np.ascontiguousarray(
                w1T.reshape(KB_, 128, IT_, 128).transpose(2, 1, 0, 3)
                .astype(bf16)),
            "w3R": np.ascontiguousarray(
                w3T.reshape(KB_, 128, IT_, 128).transpose(2, 1, 0, 3)
                .astype(bf16)),
            "w2R": np.ascontiguousarray(
                w2[:, IS * c:IS * (c + 1)].T.reshape(IT_, 128, HID)
                .transpose(1, 0, 2).astype(bf16)),
        })
    return in_maps


def run(inputs, trace=False):
    """Returns (output, BassKernelResults)."""
    from concourse import bass_utils
    nc = _get_compiled()
    in_maps = _prep_inputs(inputs)
    res = bass_utils.run_bass_kernel_spmd(
        nc, in_maps, core_ids=list(range(NC)), trace=trace)
    out = np.concatenate([res.results[c]["out_own"] for c in range(NC)], axis=0)
    return out.astype(np.float32), res


def kernel(**inputs):
    out, _ = run(inputs)
    return out



# revision 8
# speedup vs baseline: 1.1146x; 1.1146x over previous
"""InternLM2 decoder layer on 8 trn2 NeuronCores, tensor-parallel (bass/Tile).

Self-contained: hardcodes shapes/sharding. Host shards + pre-tiles weights
(bf16, RMSNorm gammas folded into consuming matmul weights), device computes
the layer, host reassembles the output.

Per-core sharding: q-heads 4c..4c+3 + kv-head c (GQA groups align), wo/w2
row-sharded, w1/w3 col-sharded. Collectives are chunked for overlap:
AG1/AG2 split in 2 hid-halves (matmul k-chains start on half 0), RS1/RS2
split in 2 token-halves (fire mid-compute). RS output ownership: core c
owns 128-token pieces {1024g + 128c : g=0,1} for norm2/residual/output.

Dataflow: norm1(own 256 tok) -> AG(xnT halves) -> per-512-chunk
QKV/rope/attention/wo -> RS1 halves -> fused resid+norm2 -> AG2 halves ->
MLP in 512-token quarters (w1/w3/w2 streamed) -> RS2 halves -> residual.
Scores computed transposed [s, t]; softmax denominator via ones-matmul,
reciprocal on scalar engine, partition-broadcast via rank-1 PE matmul,
causal mask via precomputed bf16 mask multiply on DVE.
"""
import sys
import numpy as np
import ml_dtypes

sys.path.insert(0, "/opt/trn_rl_repo")

HID, H, K, D, INTER, T = 4096, 32, 8, 128, 14336, 2048
EPS, THETA = 1e-5, 1000000.0
NC = 8                 # cores
QH = H // NC           # q heads per core = 4
JD = QH * D            # per-core attn out dim = 512
IS = INTER // NC       # inter shard = 1792
TOK = T // NC          # owned tokens per core = 256
CH = 512               # token chunk for compute loops
NCH = T // CH          # 4
KB_ = HID // 128       # 32 k-tiles
KBH = KB_ // 2         # 16 k-tiles per hid half
IT_ = IS // 128        # 14 i-tiles
SCALE = 1.0 / np.sqrt(D)

bf16 = ml_dtypes.bfloat16

_compiled = None


def _build():
    from contextlib import ExitStack
    import concourse.bacc as bacc
    import concourse.bass as bass
    import concourse.tile as tile
    from concourse import mybir

    f32 = mybir.dt.float32
    bf = mybir.dt.bfloat16
    AF = mybir.ActivationFunctionType
    PSUM = bass.MemorySpace.PSUM

    nc = bacc.Bacc("TRN2", target_bir_lowering=False, debug=False, num_devices=NC)

    # ---- I/O (per-core shapes; weights pre-tiled on host) ----
    x_own = nc.dram_tensor("x_own", [TOK, HID], f32, kind="ExternalInput")
    x2_own = nc.dram_tensor("x2_own", [2, 128, HID], f32, kind="ExternalInput")
    cosT = nc.dram_tensor("cosT", [D // 2, T], f32, kind="ExternalInput")
    sinT = nc.dram_tensor("sinT", [D // 2, T], f32, kind="ExternalInput")
    ident = nc.dram_tensor("ident", [128, 128], bf, kind="ExternalInput")
    masksI = nc.dram_tensor("masksI", [128, 4, CH], bf, kind="ExternalInput")
    wqkvR = nc.dram_tensor("wqkvR", [128, KB_, JD + 2 * D], bf, kind="ExternalInput")
    woR = nc.dram_tensor("woR", [128, QH, HID], bf, kind="ExternalInput")
    w1R = nc.dram_tensor("w1R", [IT_, 128, KB_, 128], bf, kind="ExternalInput")
    w3R = nc.dram_tensor("w3R", [IT_, 128, KB_, 128], bf, kind="ExternalInput")
    w2R = nc.dram_tensor("w2R", [128, IT_, HID], bf, kind="ExternalInput")
    out_own = nc.dram_tensor("out_own", [2, 128, HID], f32, kind="ExternalOutput")

    # ---- internal DRAM (collective bounce + h spill) ----
    ag1_in = [nc.dram_tensor(f"ag1_in{h}", [HID // 2, TOK], bf, kind="Internal")
              for h in range(2)]
    ag1_out = [nc.dram_tensor(f"ag1_out{h}", [NC, HID // 2, TOK], bf,
                              kind="Internal", addr_space="Shared")
               for h in range(2)]
    rs1_in = nc.dram_tensor("rs1_in", [T, HID], bf, kind="Internal")
    rs1_out = [nc.dram_tensor(f"rs1_out{g}", [128, HID], bf, kind="Internal")
               for g in range(2)]
    ag2_in = [nc.dram_tensor(f"ag2_in{h}", [HID // 2, TOK], bf, kind="Internal")
              for h in range(2)]
    ag2_out = [nc.dram_tensor(f"ag2_out{h}", [NC, HID // 2, TOK], bf,
                              kind="Internal", addr_space="Shared")
               for h in range(2)]
    rs2_in = nc.dram_tensor("rs2_in", [T, HID], bf, kind="Internal")
    rs2_out = [nc.dram_tensor(f"rs2_out{g}", [128, HID], bf, kind="Internal")
               for g in range(2)]
    h_spill = nc.dram_tensor("h_spill", [2, 128, HID], f32, kind="Internal")

    RG = [list(range(NC))]

    def allgather(in_t, out_t):
        nc.gpsimd.collective_compute(
            "AllGather", mybir.AluOpType.bypass, replica_groups=RG,
            ins=[in_t.ap()], outs=[out_t.ap()])

    def reducescatter(in_ap, out_t):
        nc.gpsimd.collective_compute(
            "ReduceScatter", mybir.AluOpType.add, replica_groups=RG,
            ins=[in_ap], outs=[out_t.ap()])

    with tile.TileContext(nc) as tc, ExitStack() as top:
        const = top.enter_context(tc.tile_pool(name="const", bufs=1))
        ident_sb = const.tile([128, 128], bf)
        nc.sync.dma_start(ident_sb[:], ident.ap())
        ones_sb = const.tile([128, 1], bf)
        nc.vector.memset(ones_sb[:], 1.0)
        ones1_sb = const.tile([1, 128], f32)
        nc.vector.memset(ones1_sb[:], 1.0)
        eps_sb = const.tile([128, 1], f32)
        nc.vector.memset(eps_sb[:], EPS)
        masks_sb = const.tile([128, 4, CH], bf)
        nc.sync.dma_start(masks_sb[:], masksI.ap())
        cos_sb = const.tile([D // 2, T], f32)
        sin_sb = const.tile([D // 2, T], f32)
        nc.sync.dma_start(cos_sb[:], cosT.ap())
        nc.sync.dma_start(sin_sb[:], sinT.ap())

        # attention weights: prefetch during norm1/AG1; pool explicitly closed
        # after phase 2 so MLP-phase SBUF fits
        wctx = tc.tile_pool(name="wattn", bufs=1)
        wpool = wctx.__enter__()
        wqkv_sb = wpool.tile([128, KB_, JD + 2 * D], bf)
        nc.sync.dma_start(wqkv_sb[:], wqkvR.ap())
        wo_sb = wpool.tile([128, QH, HID], bf)
        nc.sync.dma_start(wo_sb[:], woR.ap())

        # norm of [128, HID] f32 tile (already in SBUF) -> bf16, transposed,
        # into per-half staging tiles [128, KBH, cols]
        def norm_transpose(pool, psum, ht, stage_h, col0, ncols, scratch_tag):
            sq = pool.tile([128, HID], bf, tag=scratch_tag + "sq")
            ssq = pool.tile([128, 1], f32, tag=scratch_tag + "ssq")
            nc.scalar.activation(sq[:], ht, AF.Square, accum_out=ssq[:])
            rms = pool.tile([128, 1], f32, tag=scratch_tag + "rm")
            nc.scalar.activation(rms[:], ssq[:], AF.Sqrt,
                                 scale=1.0 / HID, bias=eps_sb[:])
            rinv = pool.tile([128, 1], f32, tag=scratch_tag + "ri")
            nc.vector.reciprocal(rinv[:], rms[:])
            xn = pool.tile([128, HID], bf, tag=scratch_tag + "xn")
            nc.vector.tensor_scalar_mul(xn[:], ht, rinv[:])
            for kb in range(KB_):
                tp = psum.tile([128, 128], bf, tag=scratch_tag + "tp")
                nc.tensor.transpose(tp[:], xn[:, kb * 128:(kb + 1) * 128],
                                    ident_sb[:])
                nc.vector.tensor_copy(
                    stage_h[kb // KBH][:, kb % KBH, col0:col0 + ncols], tp[:, :ncols])

        # ================= phase 1: norm1 + AG1 (hid halves) =================
        with ExitStack() as ph, nc.named_scope("norm1"):
            pool = ph.enter_context(tc.tile_pool(name="norm1", bufs=2))
            stg_pool = ph.enter_context(tc.tile_pool(name="n1stage", bufs=1))
            psum = ph.enter_context(tc.tile_pool(name="n1ps", bufs=4, space=PSUM))
            stage = [stg_pool.tile([128, KBH, TOK], bf, tag=f"stg{h}",
                                   name=f"stg{h}") for h in range(2)]
            for b in range(TOK // 128):
                xt = pool.tile([128, HID], f32, tag="xt")
                nc.sync.dma_start(xt[:], x_own.ap()[b * 128:(b + 1) * 128, :])
                norm_transpose(pool, psum, xt[:], stage, b * 128, 128, "n1")
            for h in range(2):
                nc.sync.dma_start(
                    ag1_in[h].ap().rearrange("(a p) t -> p a t", p=128), stage[h][:])
        for h in range(2):
            allgather(ag1_in[h], ag1_out[h])

        # ============ phase 2: QKV + attention + wo, chunked RS1 ============
        with ExitStack() as ph:
            kv_pool = ph.enter_context(tc.tile_pool(name="kv", bufs=1))
            kT_sb = kv_pool.tile([128, T], bf)             # roped K, [d, s]
            v_sb = kv_pool.tile([128, T // 128, D], bf)    # [s-part, s-tile, d]

            xc_pool = ph.enter_context(tc.tile_pool(name="attnxc", bufs=1))
            ap_ = ph.enter_context(tc.tile_pool(name="attn", bufs=3))
            ps_acc = ph.enter_context(tc.tile_pool(name="accps", bufs=2, space=PSUM))
            ps_sc = ph.enter_context(tc.tile_pool(name="scps", bufs=2, space=PSUM))
            ps_pv = ph.enter_context(tc.tile_pool(name="pvps", bufs=2, space=PSUM))
            ps_sm = ph.enter_context(tc.tile_pool(name="smps", bufs=1, space=PSUM))

            def rope(dst, src, t0):
                c = cos_sb[:, t0:t0 + CH]
                s = sin_sb[:, t0:t0 + CH]
                t1 = ap_.tile([64, CH], f32, tag="rp1")
                t2 = ap_.tile([64, CH], f32, tag="rp2")
                nc.vector.tensor_mul(t1[:], src[0:64, :], c)
                nc.vector.tensor_mul(t2[:], src[64:128, :], s)
                nc.vector.tensor_sub(dst[0:64, :], t1[:], t2[:])
                nc.vector.tensor_mul(t1[:], src[64:128, :], c)
                nc.vector.tensor_mul(t2[:], src[0:64, :], s)
                nc.vector.tensor_add(dst[64:128, :], t1[:], t2[:])

            for j in range(NCH):
                t0 = j * CH
                with nc.named_scope(f"attn{j}"):
                    xc = xc_pool.tile([128, KB_, CH], bf, tag="xc")
                    for h in range(2):
                        for ci, c in enumerate((2 * j, 2 * j + 1)):
                            nc.sync.dma_start(
                                xc[:, h * KBH:(h + 1) * KBH,
                                   ci * 256:(ci + 1) * 256],
                                ag1_out[h].ap()[c].rearrange(
                                    "(a p) t -> p a t", p=128))
                    qT = ap_.tile([128, QH, CH], bf, tag="qT")
                    for m in range(6):
                        acc = ps_acc.tile([128, CH], f32, tag="acc")
                        for kb in range(KB_):
                            nc.tensor.matmul(
                                acc[:],
                                wqkv_sb[:, kb, m * 128:(m + 1) * 128],
                                xc[:, kb, :],
                                start=(kb == 0), stop=(kb == KB_ - 1))
                        if m < QH:
                            rope(qT[:, m, :], acc, t0)
                        elif m == QH:
                            rope(kT_sb[:, t0:t0 + CH], acc, t0)
                        else:
                            vb = ap_.tile([128, CH], bf, tag="vb")
                            nc.vector.tensor_copy(vb[:], acc[:])
                            for sb_ in range(CH // 128):
                                tp = ps_sc.tile([128, 128], bf, tag="sc")
                                nc.tensor.transpose(
                                    tp[:], vb[:, sb_ * 128:(sb_ + 1) * 128],
                                    ident_sb[:])
                                nc.vector.tensor_copy(
                                    v_sb[:, 4 * j + sb_, :], tp[:])

                    aoT = ap_.tile([128, QH, CH], bf, tag="aoT")
                    ns = 4 * j + 4
                    for hq in range(QH):
                        pv = ps_pv.tile([128, CH], f32, tag="pv")
                        den = ps_sm.tile([1, CH], f32, tag="den")
                        for si in range(ns):
                            sc = ps_sc.tile([128, CH], f32, tag="sc")
                            nc.tensor.matmul(
                                sc[:], kT_sb[:, si * 128:(si + 1) * 128],
                                qT[:, hq, :], start=True, stop=True)
                            pT = ap_.tile([128, CH], bf, tag="pT")
                            nc.scalar.activation(pT[:], sc[:], AF.Exp, scale=SCALE)
                            if si >= 4 * j:          # diagonal: zero s > t
                                pm = ap_.tile([128, CH], bf, tag="pm")
                                nc.vector.tensor_mul(
                                    pm[:], pT[:], masks_sb[:, si - 4 * j, :])
                                pT = pm
                            nc.tensor.matmul(pv[:], v_sb[:, si, :], pT[:],
                                             start=(si == 0), stop=(si == ns - 1))
                            nc.tensor.matmul(den[:], ones_sb[:], pT[:],
                                             start=(si == 0), stop=(si == ns - 1))
                        rec = ap_.tile([1, CH], f32, tag="rec")
                        nc.vector.reciprocal(rec[:], den[:])
                        recb = ps_sm.tile([128, CH], f32, tag="recb")
                        nc.tensor.matmul(recb[:], ones1_sb[:], rec[:],
                                         start=True, stop=True)
                        rcs = ap_.tile([128, CH], f32, tag="rcs")
                        nc.vector.tensor_copy(rcs[:], recb[:])
                        nc.vector.tensor_mul(aoT[:, hq, :], pv[:], rcs[:])

                    # wo: out[t, hid] rows t0+128m, K=512 over heads
                    for m in range(CH // 128):
                        for nh in range(8):
                            acc = ps_acc.tile([128, 512], f32, tag="acc")
                            for kb in range(QH):
                                nc.tensor.matmul(
                                    acc[:],
                                    aoT[:, kb, m * 128:(m + 1) * 128],
                                    wo_sb[:, kb, nh * 512:(nh + 1) * 512],
                                    start=(kb == 0), stop=(kb == QH - 1))
                            ob = ap_.tile([128, 512], bf, tag="ob")
                            nc.vector.tensor_copy(ob[:], acc[:])
                            nc.sync.dma_start(
                                rs1_in.ap()[t0 + m * 128: t0 + (m + 1) * 128,
                                            nh * 512:(nh + 1) * 512], ob[:])
                if j % 2 == 1:
                    reducescatter(rs1_in.ap()[(j - 1) * CH:(j + 1) * CH, :],
                                  rs1_out[j // 2])
        wctx.__exit__(None, None, None)

        # ======== phase 3: fused h = x + rs1, norm2 (per 128-piece) ========
        with ExitStack() as ph:
            pool = ph.enter_context(tc.tile_pool(name="norm2", bufs=2))
            stg_pool = ph.enter_context(tc.tile_pool(name="n2stage", bufs=1))
            psum = ph.enter_context(tc.tile_pool(name="n2ps", bufs=4, space=PSUM))
            stage = [stg_pool.tile([128, KBH, TOK], bf, tag=f"stg2{h}",
                                   name=f"stg2{h}") for h in range(2)]
            for g in range(2):
                with nc.named_scope(f"norm2_{g}"):
                    xt = pool.tile([128, HID], f32, tag="xt")
                    nc.sync.dma_start(xt[:], x2_own.ap()[g])
                    rt = pool.tile([128, HID], bf, tag="rt")
                    nc.sync.dma_start(rt[:], rs1_out[g].ap())
                    ht = pool.tile([128, HID], f32, tag="ht")
                    nc.vector.tensor_add(ht[:], xt[:], rt[:])
                    nc.sync.dma_start(h_spill.ap()[g], ht[:])
                    norm_transpose(pool, psum, ht[:], stage, g * 128, 128, "n2")
            for h in range(2):
                nc.sync.dma_start(
                    ag2_in[h].ap().rearrange("(a p) t -> p a t", p=128), stage[h][:])
        for h in range(2):
            allgather(ag2_in[h], ag2_out[h])

        # ============== phase 4: MLP in 512-token quarters ==============
        with ExitStack() as ph:
            mxc = ph.enter_context(tc.tile_pool(name="mlpxc", bufs=2))
            mact = ph.enter_context(tc.tile_pool(name="mlpact", bufs=2))
            mw = ph.enter_context(tc.tile_pool(name="mlpw", bufs=2))
            mw2 = ph.enter_context(tc.tile_pool(name="mlpw2", bufs=2))
            msc = ph.enter_context(tc.tile_pool(name="mlpsc", bufs=3))
            ps_g = ph.enter_context(tc.tile_pool(name="gups", bufs=2, space=PSUM))
            ps_d = ph.enter_context(tc.tile_pool(name="dps", bufs=2, space=PSUM))

            for q in range(NCH):
                t0 = q * CH
                g = q // 2
                with nc.named_scope(f"mlp{q}"):
                    xc = mxc.tile([128, KB_, CH], bf, tag="xcq")
                    for h in range(2):
                        for ci in range(4):
                            c = 4 * (q % 2) + ci
                            nc.sync.dma_start(
                                xc[:, h * KBH:(h + 1) * KBH,
                                   ci * 128:(ci + 1) * 128],
                                ag2_out[h].ap()[c].rearrange(
                                    "(a p) t -> p a t", p=128)[:, :,
                                    g * 128:(g + 1) * 128])
                    actT = mact.tile([128, IT_, CH], bf, tag="actT")
                    for it in range(IT_):
                        w1t = mw.tile([128, KB_, 128], bf, tag="w1t")
                        w3t = mw.tile([128, KB_, 128], bf, tag="w3t")
                        nc.sync.dma_start(w1t[:], w1R.ap()[it])
                        nc.sync.dma_start(w3t[:], w3R.ap()[it])
                        gp = ps_g.tile([128, CH], f32, tag="g")
                        up = ps_g.tile([128, CH], f32, tag="u")
                        for kb in range(KB_):
                            nc.tensor.matmul(gp[:], w1t[:, kb, :], xc[:, kb, :],
                                             start=(kb == 0), stop=(kb == KB_ - 1))
                        for kb in range(KB_):
                            nc.tensor.matmul(up[:], w3t[:, kb, :], xc[:, kb, :],
                                             start=(kb == 0), stop=(kb == KB_ - 1))
                        sg = msc.tile([128, CH], f32, tag="sg")
                        nc.scalar.activation(sg[:], gp[:], AF.Silu)
                        nc.vector.tensor_mul(actT[:, it, :], sg[:], up[:])
                    # down-proj: stream w2 col slices, contract over it
                    for s8 in range(8):
                        w2s = mw2.tile([128, IT_, 512], bf, tag="w2s")
                        nc.sync.dma_start(
                            w2s[:], w2R.ap()[:, :, s8 * 512:(s8 + 1) * 512])
                        for m in range(CH // 128):
                            acc = ps_d.tile([128, 512], f32, tag="d")
                            for it in range(IT_):
                                nc.tensor.matmul(
                                    acc[:], actT[:, it, m * 128:(m + 1) * 128],
                                    w2s[:, it, :],
                                    start=(it == 0), stop=(it == IT_ - 1))
                            ob = msc.tile([128, 512], bf, tag="ob")
                            nc.vector.tensor_copy(ob[:], acc[:])
                            nc.sync.dma_start(
                                rs2_in.ap()[t0 + m * 128: t0 + (m + 1) * 128,
                                            s8 * 512:(s8 + 1) * 512], ob[:])
                if q % 2 == 1:
                    reducescatter(rs2_in.ap()[(q - 1) * CH:(q + 1) * CH, :],
                                  rs2_out[q // 2])

        # ================= phase 5: final residual =================
        with ExitStack() as ph, nc.named_scope("fin"):
            pool = ph.enter_context(tc.tile_pool(name="fin", bufs=2))
            for g in range(2):
                ht = pool.tile([128, HID], f32, tag="ht")
                nc.sync.dma_start(ht[:], h_spill.ap()[g])
                rt = pool.tile([128, HID], bf, tag="rt")
                nc.sync.dma_start(rt[:], rs2_out[g].ap())
                ot = pool.tile([128, HID], f32, tag="ot")
                nc.vector.tensor_add(ot[:], ht[:], rt[:])
                nc.sync.dma_start(out_own.ap()[g], ot[:])

    nc.compile()
    return nc


def _get_compiled():
    global _compiled
    if _compiled is None:
        _compiled = _build()
    return _compiled


def _prep_inputs(inputs):
    x = np.asarray(inputs["hidden_states"], np.float32)
    pos = np.asarray(inputs["position_ids"]).astype(np.float32)
    wqkv = np.asarray(inputs["wqkv"], np.float32)
    wo = np.asarray(inputs["wo"], np.float32)
    w1 = np.asarray(inputs["w1"], np.float32)
    w3 = np.asarray(inputs["w3"], np.float32)
    w2 = np.asarray(inputs["w2"], np.float32)
    anw = np.asarray(inputs["attn_norm_w"], np.float32)
    fnw = np.asarray(inputs["ffn_norm_w"], np.float32)

    inv_freq = 1.0 / (THETA ** (np.arange(0, D, 2, dtype=np.float32) / D))
    freqs = pos[:, None] * inv_freq
    cosT_np = np.ascontiguousarray(np.cos(freqs).T.astype(np.float32))
    sinT_np = np.ascontiguousarray(np.sin(freqs).T.astype(np.float32))
    ident_np = np.ascontiguousarray(np.eye(128, dtype=bf16))

    # causal masks for diagonal tiles: masks[p, r, f] = (f >= 128*r + p)
    p_ = np.arange(128)[:, None, None]
    r_ = np.arange(4)[None, :, None]
    f_ = np.arange(CH)[None, None, :]
    masks_np = np.ascontiguousarray((f_ >= 128 * r_ + p_).astype(bf16))

    wqkv_f = wqkv * anw[None, :]
    w1_f = w1 * fnw[None, :]
    w3_f = w3 * fnw[None, :]

    def ktile_major(wT, n):           # [HID, n] -> [128, KB_, n]
        return np.ascontiguousarray(
            wT.reshape(KB_, 128, n).transpose(1, 0, 2).astype(bf16))

    in_maps = []
    for c in range(NC):
        qrows = np.arange(JD * c, JD * (c + 1))
        krows = H * D + np.arange(D * c, D * (c + 1))
        vrows = (H + K) * D + np.arange(D * c, D * (c + 1))
        rows = np.concatenate([qrows, krows, vrows])
        w1T = w1_f[IS * c:IS * (c + 1)].T          # [HID, IS]
        w3T = w3_f[IS * c:IS * (c + 1)].T
        x2 = np.stack([x[1024 * g + 128 * c: 1024 * g + 128 * (c + 1)]
                       for g in range(2)])
        in_maps.append({
            "x_own": np.ascontiguousarray(x[TOK * c:TOK * (c + 1)]),
            "x2_own": np.ascontiguousarray(x2),
            "cosT": cosT_np, "sinT": sinT_np, "ident": ident_np,
            "masksI": masks_np,
            "wqkvR": ktile_major(wqkv_f[rows].T, JD + 2 * D),
            "woR": np.ascontiguousarray(
                wo[:, JD * c:JD * (c + 1)].T.reshape(QH, 128, HID)
                .transpose(1, 0, 2).astype(bf16)),
            "w1R": np.ascontiguousarray(
                w1T.reshape(KB_, 128, IT_, 128).transpose(2, 1, 0, 3)
                .astype(bf16)),
            "w3R": np.ascontiguousarray(
                w3T.reshape(KB_, 128, IT_, 128).transpose(2, 1, 0, 3)
                .astype(bf16)),
            "w2R": np.ascontiguousarray(
                w2[:, IS * c:IS * (c + 1)].T.reshape(IT_, 128, HID)
                .transpose(1, 0, 2).astype(bf16)),
        })
    return in_maps


def run(inputs, trace=False):
    """Returns (output, BassKernelResults)."""
    from concourse import bass_utils
    nc = _get_compiled()
    in_maps = _prep_inputs(inputs)
    res = bass_utils.run_bass_kernel_spmd(
        nc, in_maps, core_ids=list(range(NC)), trace=trace)
    out = np.empty((T, HID), np.float32)
    for c in range(NC):
        for g in range(2):
            out[1024 * g + 128 * c: 1024 * g + 128 * (c + 1)] = \
                res.results[c]["out_own"][g]
    return out, res


def kernel(**inputs):
    out, _ = run(inputs)
    return out
